# revision 1
# baseline (speedup 1.0000x reference)
"""Trainium2 Bass kernel for multi-head attention (B=4, H=8, L=2048, dim=512).

Sharding: 8 cores = 4 batches x 2 sequence halves. Each core computes the
full attention output for one batch's 1024-query half (all 8 heads), using
K/V over the full 2048-key sequence; the output projection contracts the
full hidden dim locally, so no cross-core communication is needed.

Per-core pipeline (all matmuls fp16 x fp16 -> fp32 PSUM):
  1. QKV projections: Q [hc, q], K [hc, k], V^T [k, hc] (V^T produced
     directly by using x as the stationary operand).
  2. Per head pair (heads 2m, 2m+1 live on partitions 0-63 / 64-127):
     scores S^T[k, q] = K_h^T Q_h via K=64 row-packed matmuls
     (tile_position (0,0)/(64,0)), exp on ScalarE PSUM->SBUF (no max
     subtraction needed: |s| <= ~7), attn@V via col-packed matmuls
     (tile_position (0,0)/(0,64)) accumulating O^T[c, q] pairs stacked
     [128, q]. Softmax denominators: incremental pairwise adds on VectorE
     into an 8-slot partial buffer, folded once, then summed across
     partitions by an accumulating ones-column matmul; normalization is
     fused into the PSUM->SBUF copy of O^T.
  3. Output projection + bias.
"""
import numpy as np

import concourse.bass as bass
import concourse.tile as tile
from concourse import bacc, mybir
from concourse.bass_utils import run_bass_kernel_spmd

F16 = mybir.dt.float16
F32 = mybir.dt.float32
P = 128
D = 512          # model dim
L = 2048         # full sequence (keys)
QL = 1024        # per-core query length
H = 8            # heads
C = 64           # head dim
HID = 512        # H * C
DC = D // P      # 4 contraction chunks
KT = L // P      # 16 key tiles
N = 512          # matmul free-dim chunk
QC = QL // N     # 2 query chunks
LC = L // N      # 4 key free-dim chunks
SCALE = C ** -0.5
EXP = mybir.ActivationFunctionType.Exp


def emit(nc, tc, x, wq, wk, wv, wo, bias, out):
    import contextlib
    ctx = contextlib.ExitStack()
    with ctx:
        # ---- pools -----------------------------------------------------
        consts = ctx.enter_context(tc.tile_pool(name="consts", bufs=1))
        qkv = ctx.enter_context(tc.tile_pool(name="qkv", bufs=1))
        ph1 = ctx.enter_context(tc.tile_pool(name="ph1", bufs=1))
        atp = ctx.enter_context(tc.tile_pool(name="atp", bufs=10))
        t8p = ctx.enter_context(tc.tile_pool(name="t8p", bufs=2))
        t4p = ctx.enter_context(tc.tile_pool(name="t4p", bufs=2))
        t2p = ctx.enter_context(tc.tile_pool(name="t2p", bufs=2))
        t1p = ctx.enter_context(tc.tile_pool(name="t1p", bufs=2))
        rbp = ctx.enter_context(tc.tile_pool(name="rbp", bufs=1))
        otup = ctx.enter_context(tc.tile_pool(name="otup", bufs=2))
        outp = ctx.enter_context(tc.tile_pool(name="outp", bufs=2))
        # PSUM: ps 2x2 banks + po 1x2 + pss 2x1 = 8 banks. The projection
        # phases borrow the po/pss slots so the scores slots stay free.
        pp_s = ctx.enter_context(tc.tile_pool(name="pps", bufs=2, space="PSUM"))
        pp_o = ctx.enter_context(tc.tile_pool(name="ppo", bufs=1, space="PSUM"))
        pp_sum = ctx.enter_context(tc.tile_pool(name="ppsum", bufs=2, space="PSUM"))
        _prj = [0]

        def proj_psum():
            _prj[0] += 1
            if _prj[0] % 2 == 0:
                return pp_o.tile([P, N], F32, tag="po", name=f"prj{_prj[0]}")
            return pp_sum.tile([P, N], F32, tag="pss", name=f"prj{_prj[0]}")

        # ---- persistent SBUF ------------------------------------------
        wo_sb = consts.tile([P, DC, HID], F16)
        bias_sb = consts.tile([P, DC], F32)
        ones_sb = consts.tile([P, C], F16)
        nc.vector.memset(ones_sb[:], 1.0)

        q_sb = qkv.tile([P, DC, QL], F16)
        k_sb = qkv.tile([P, DC, L], F16)
        vt_sb = qkv.tile([P, KT, HID], F16)
        ot_sb = qkv.tile([P, DC, QL], F16)

        # ---- phase 1: load x + weights, QKV projections ----------------
        x_sb = ph1.tile([P, DC, L], F16)
        wq_sb = ph1.tile([P, DC, HID], F16)
        wk_sb = ph1.tile([P, DC, HID], F16)
        wv_sb = ph1.tile([P, DC, HID], F16)
        xr = x.rearrange("(a p) n -> p a n", p=P)
        # query-half of x + wq first so the Q projection starts ASAP
        nc.sync.dma_start(out=wq_sb[:], in_=wq.rearrange("(a p) n -> p a n", p=P))
        nc.sync.dma_start(out=x_sb[:, :, 0:N], in_=xr[:, :, 0:N])
        nc.sync.dma_start(out=x_sb[:, :, N:QL], in_=xr[:, :, N:QL])
        nc.sync.dma_start(out=wk_sb[:], in_=wk.rearrange("(a p) n -> p a n", p=P))
        nc.sync.dma_start(out=x_sb[:, :, QL:L], in_=xr[:, :, QL:L])
        nc.sync.dma_start(out=wv_sb[:], in_=wv.rearrange("(a p) n -> p a n", p=P))
        nc.sync.dma_start(out=wo_sb[:], in_=wo.rearrange("(a p) n -> p a n", p=P))
        nc.sync.dma_start(out=bias_sb[:], in_=bias)

        # Q: [hc, q] = wq^T @ xq  (queries are x cols 0:QL)
        for m in range(DC):
            for qc in range(QC):
                ps = proj_psum()
                for dc in range(DC):
                    nc.tensor.matmul(
                        ps[:],
                        lhsT=wq_sb[:, dc, m * P:(m + 1) * P],
                        rhs=x_sb[:, dc, qc * N:(qc + 1) * N],
                        start=(dc == 0), stop=(dc == DC - 1),
                    )
                nc.vector.tensor_copy(q_sb[:, m, qc * N:(qc + 1) * N], ps[:])
        # K: [hc, k]
        for m in range(DC):
            for lc in range(LC):
                ps = proj_psum()
                for dc in range(DC):
                    nc.tensor.matmul(
                        ps[:],
                        lhsT=wk_sb[:, dc, m * P:(m + 1) * P],
                        rhs=x_sb[:, dc, lc * N:(lc + 1) * N],
                        start=(dc == 0), stop=(dc == DC - 1),
                    )
                nc.vector.tensor_copy(k_sb[:, m, lc * N:(lc + 1) * N], ps[:])

        def vt_proj(kt):
            # V^T: [k, hc] (x stationary); interleaved into pair-0's kt loop
            # so attention exps start while the projection is still running.
            # Uses only the pp_sum slots: pair-0's po tile owns pp_o by then.
            ps = pp_sum.tile([P, N], F32, tag="pss", name=f"vtp{kt}")
            for dc in range(DC):
                nc.tensor.matmul(
                    ps[:],
                    lhsT=x_sb[:, dc, kt * P:(kt + 1) * P],
                    rhs=wv_sb[:, dc, :],
                    start=(dc == 0), stop=(dc == DC - 1),
                )
            nc.vector.tensor_copy(vt_sb[:, kt, :], ps[:])

        def emit_l1(kt, at_prev, at_A, at_B, t8_A, t8_B, t4_A, t4_B):
            j = kt // 2
            nc.vector.tensor_add(t8_A[:, j, :], at_prev[0][:], at_A[:])
            nc.vector.tensor_add(t8_B[:, j, :], at_prev[1][:], at_B[:])
            if kt % 4 == 3:
                # eager fold 2 fresh t8 slots into t4 to keep the
                # end-of-pair reduction tail short
                i = kt // 4
                nc.vector.tensor_add(
                    t4_A[:, i, :], t8_A[:, 2 * i, :], t8_A[:, 2 * i + 1, :]
                )
                nc.vector.tensor_add(
                    t4_B[:, i, :], t8_B[:, 2 * i, :], t8_B[:, 2 * i + 1, :]
                )

        def finish_tail(st):
            # denominators part 2: partition-sum via the all-ones [128, 64]
            # stationary matmul (output rows all equal the sum, landing
            # pre-broadcast and col-positioned per head half), reciprocal,
            # then the deferred normalization multiply. Deferred into the
            # NEXT pair's kt2 so these matmuls never sit in the in-order PE
            # queue waiting on the fold chain.
            m, otu, t1s = st
            rbr = rbp.tile([P, QL], F32, tag="rbr", name=f"rbr{m}")
            for qc in range(QC):
                rb_ps = pp_sum.tile([P, N], F32, tag="pss", name=f"rb{m}_{qc}")
                for half in range(2):
                    nc.tensor.matmul(
                        rb_ps[half * C:(half + 1) * C, :],
                        lhsT=ones_sb[:],
                        rhs=t1s[half][:, qc * N:(qc + 1) * N],
                        start=True, stop=True,
                        tile_position=(0, half * C), skip_group_check=True,
                    )
                nc.vector.reciprocal_approx_fast(
                    out=rbr[:, qc * N:(qc + 1) * N], in_=rb_ps[:]
                )
            nc.vector.tensor_mul(ot_sb[:, m, :], otu[:], rbr[:])

        pending = None
        # ---- phase 2: attention, one head pair (2m, 2m+1) at a time ----
        for m in range(DC):
            po = pp_o.tile([P, QL], F32, tag="po", name=f"po{m}")
            t8_A = t8p.tile([P, KT // 2, QL], F16, tag="t8", name=f"t8a{m}")
            t8_B = t8p.tile([P, KT // 2, QL], F16, tag="t8", name=f"t8b{m}")
            t4_A = t4p.tile([P, KT // 4, QL], F16, tag="t4", name=f"t4a{m}")
            t4_B = t4p.tile([P, KT // 4, QL], F16, tag="t4", name=f"t4b{m}")
            at_prev = [None, None]  # odd-kt pairing for the L1 adds
            for kt in range(KT):
                if m == 0:
                    vt_proj(kt)
                if pending is not None and kt == 2:
                    finish_tail(pending)
                    pending = None
                ps_A = pp_s.tile([P, QL], F32, tag="ps")
                ps_B = pp_s.tile([P, QL], F32, tag="ps")
                at_A = atp.tile([P, QL], F16, tag="at")
                at_B = atp.tile([P, QL], F16, tag="at")
                for qc in range(QC):
                    nc.tensor.matmul(
                        ps_A[:, qc * N:(qc + 1) * N],
                        lhsT=k_sb[0:C, m, kt * P:(kt + 1) * P],
                        rhs=q_sb[0:C, m, qc * N:(qc + 1) * N],
                        start=True, stop=True, tile_position=(0, 0),
                    )
                for qc in range(QC):
                    nc.tensor.matmul(
                        ps_B[:, qc * N:(qc + 1) * N],
                        lhsT=k_sb[C:P, m, kt * P:(kt + 1) * P],
                        rhs=q_sb[C:P, m, qc * N:(qc + 1) * N],
                        start=True, stop=True, tile_position=(C, 0),
                    )
                nc.scalar.activation(at_A[:], ps_A[:], EXP)
                nc.scalar.activation(at_B[:], ps_B[:], EXP)
                # attn @ V: col-packed pair, accumulate over kt. The two
                # heads accumulate into disjoint partition halves of the
                # same banks; has_written is per element so the region-level
                # group check is safely skipped.
                for qc in range(QC):
                    nc.tensor.matmul(
                        po[0:C, qc * N:(qc + 1) * N],
                        lhsT=vt_sb[:, kt, (2 * m) * C:(2 * m + 1) * C],
                        rhs=at_A[:, qc * N:(qc + 1) * N],
                        start=(kt == 0), stop=(kt == KT - 1),
                        tile_position=(0, 0), skip_group_check=True,
                    )
                    nc.tensor.matmul(
                        po[C:P, qc * N:(qc + 1) * N],
                        lhsT=vt_sb[:, kt, (2 * m + 1) * C:(2 * m + 2) * C],
                        rhs=at_B[:, qc * N:(qc + 1) * N],
                        start=(kt == 0), stop=(kt == KT - 1),
                        tile_position=(0, C), skip_group_check=True,
                    )
                if kt % 2 == 0:
                    at_prev = [at_A, at_B]
                    if m == DC - 1 and kt == 8:
                        # last pair: eager-fold the first t2 halves so the
                        # final denominator chain (which gates the output
                        # projection) is shorter
                        t2_A3 = t2p.tile([P, KT // 8, QL], F16, tag="t2",
                                         name="t2a3")
                        t2_B3 = t2p.tile([P, KT // 8, QL], F16, tag="t2",
                                         name="t2b3")
                        nc.vector.tensor_add(
                            t2_A3[:, 0, :], t4_A[:, 0, :], t4_A[:, 1, :]
                        )
                        nc.vector.tensor_add(
                            t2_B3[:, 0, :], t4_B[:, 0, :], t4_B[:, 1, :]
                        )
                elif kt < KT - 1:
                    emit_l1(kt, at_prev, at_A, at_B, t8_A, t8_B, t4_A, t4_B)
                else:
                    last_l1 = (kt, at_prev, at_A, at_B, t8_A, t8_B, t4_A, t4_B)

            otu = otup.tile([P, QL], F16, tag="otu", name=f"otu{m}")
            nc.vector.tensor_copy(otu[:], po[:])
            emit_l1(*last_l1)

            # denominators part 1: fold t4 -> t2 -> t1 on DVE
            t1s = []
            if m == DC - 1:
                for t4_t, t2_t in ((t4_A, t2_A3), (t4_B, t2_B3)):
                    nc.vector.tensor_add(
                        t2_t[:, 1, :], t4_t[:, 2, :], t4_t[:, 3, :]
                    )
                    t1 = t1p.tile([P, QL], F16, tag="t1")
                    nc.vector.tensor_add(t1[:], t2_t[:, 0, :], t2_t[:, 1, :])
                    t1s.append(t1)
            else:
                for t4_t in (t4_A, t4_B):
                    r4 = t4_t[:].rearrange("p (a b) q -> p a b q", a=2)
                    t2 = t2p.tile([P, KT // 8, QL], F16, tag="t2")
                    nc.vector.tensor_add(t2[:], r4[:, 0], r4[:, 1])
                    t1 = t1p.tile([P, QL], F16, tag="t1")
                    nc.vector.tensor_add(t1[:], t2[:, 0, :], t2[:, 1, :])
                    t1s.append(t1)
            pending = (m, otu, t1s)

        finish_tail(pending)

        # ---- phase 3: output projection + bias -------------------------
        for mo in range(DC):
            for qc in range(QC):
                ps = pp_s.tile([P, QL], F32, tag="ps", name=f"po3_{mo}_{qc}")
                for mh in range(DC):
                    nc.tensor.matmul(
                        ps[:, 0:N],
                        lhsT=wo_sb[:, mh, mo * P:(mo + 1) * P],
                        rhs=ot_sb[:, mh, qc * N:(qc + 1) * N],
                        start=(mh == 0), stop=(mh == DC - 1),
                    )
                ob = outp.tile([P, N], F32, tag="ob")
                nc.vector.tensor_scalar_add(ob[:], ps[:, 0:N], bias_sb[:, mo:mo + 1])
                nc.sync.dma_start(
                    out=out[mo * P:(mo + 1) * P, qc * N:(qc + 1) * N], in_=ob[:]
                )


def build():
    nc = bacc.Bacc("TRN2", target_bir_lowering=False, debug=False)
    x = nc.dram_tensor("x", [D, L], F16, kind="ExternalInput").ap()
    wq = nc.dram_tensor("wq", [D, HID], F16, kind="ExternalInput").ap()
    wk = nc.dram_tensor("wk", [D, HID], F16, kind="ExternalInput").ap()
    wv = nc.dram_tensor("wv", [D, HID], F16, kind="ExternalInput").ap()
    wo = nc.dram_tensor("wo", [HID, D], F16, kind="ExternalInput").ap()
    bias = nc.dram_tensor("bias", [P, DC], F32, kind="ExternalInput").ap()
    out = nc.dram_tensor("out", [D, QL], F32, kind="ExternalOutput").ap()
    with tile.TileContext(nc) as tc:
        emit(nc, tc, x, wq, wk, wv, wo, bias, out)
    nc.compile()
    return nc


_NC_CACHE = None


def _get_nc():
    global _NC_CACHE
    if _NC_CACHE is None:
        _NC_CACHE = build()
    return _NC_CACHE


def make_in_maps(x, w_qkv, w_out, b_out):
    """Host-side sharding: returns the 8 per-core input dicts."""
    f16 = np.float16
    wq_t = np.ascontiguousarray((w_qkv[0:HID] * SCALE).T).astype(f16)
    wk_t = np.ascontiguousarray(w_qkv[HID:2 * HID].T).astype(f16)
    wv_t = np.ascontiguousarray(w_qkv[2 * HID:3 * HID].T).astype(f16)
    wo_t = np.ascontiguousarray(w_out.T).astype(f16)
    bias = np.ascontiguousarray(b_out.reshape(DC, P).T).astype(np.float32)
    in_maps = []
    for core in range(8):
        b, halfq = core // 2, core % 2
        # rotate so this core's query half sits at columns 0:QL; key order
        # is irrelevant (softmax sums over all keys).
        x_rot = np.roll(x[b], -halfq * QL, axis=1).astype(f16)
        in_maps.append({
            "x": np.ascontiguousarray(x_rot),
            "wq": wq_t, "wk": wk_t, "wv": wv_t, "wo": wo_t,
            "bias": bias,
        })
    return in_maps


def assemble(results):
    out = np.zeros((4, D, L), np.float32)
    for core in range(8):
        b, halfq = core // 2, core % 2
        out[b][:, halfq * QL:(halfq + 1) * QL] = results[core]["out"]
    return out


def kernel(x, w_qkv, w_out, b_out):
    x = np.asarray(x, np.float32)
    w_qkv = np.asarray(w_qkv, np.float32)
    w_out = np.asarray(w_out, np.float32)
    b_out = np.asarray(b_out, np.float32)
    nc = _get_nc()
    in_maps = make_in_maps(x, w_qkv, w_out, b_out)
    res = run_bass_kernel_spmd(nc, in_maps, core_ids=list(range(8)))
    return assemble(res.results)



# revision 5
# speedup vs baseline: 1.1911x; 1.1911x over previous
"""Trainium2 Bass kernel for multi-head attention (B=4, H=8, L=2048, dim=512).

Sharding: 8 cores = 4 batches x 2 sequence halves. Each core computes the
full attention output for one batch's 1024-query half (all 8 heads), using
K/V over the full 2048-key sequence; the output projection contracts the
full hidden dim locally, so no cross-core communication is needed.

v2 redesign (per-core), targeting 3-engine balance:
  - Scores matmuls row-pack the two heads of a pair and are ISSUE-ORDERED
    A-qc0, B-qc0, A-qc1, B-qc1 so the row-disjoint pairs run concurrently
    in the PE array (pc-monotone starts mean same-row-group matmuls issued
    back-to-back serialize).
  - exp is split across engines: track A (and a subset of track B tiles)
    uses the exact ScalarE Exp; the rest of track B uses a Schraudolph
    bit-trick exp on the DVE: q is pre-scaled by 1024*log2(e) on the host,
    so one tensor_scalar add of the fp16 exponent bias with an int16
    convert, bitcast to fp16, yields exp with ~1.8% rms error (final
    output rel err ~5e-3, gate is 2e-2). ScalarE undoes the scaling with
    its free activation scale (ln2/1024).
  - Softmax denominators: track A is summed on the PE by an all-ones
    [128,64] stationary matmul accumulating [64,512] per query chunk into
    dedicated PSUM banks (broadcast across 64 partitions for free);
    track B is summed by progressive fp16 accumulators (DVE for the
    Schraudolph tiles, GpSimd for the ScalarE tiles), then one
    ones-matmul partition-sum into the same den banks at partitions
    64:128, so the normalization multiplies are partition-aligned.
  - PSUM: scores 2x2 banks, po 2, den 2 = 8 banks; phase 1/3 borrow slots.
"""
import numpy as np

import concourse.bass as bass
import concourse.tile as tile
from concourse import bacc, mybir
from concourse.bass_utils import run_bass_kernel_spmd

F16 = mybir.dt.float16
F32 = mybir.dt.float32
I16 = mybir.dt.int16
P = 128
D = 512          # model dim
L = 2048         # full sequence (keys)
QL = 1024        # per-core query length
H = 8            # heads
C = 64           # head dim
HID = 512        # H * C
DC = D // P      # 4 contraction chunks
KT = L // P      # 16 key tiles
N = 512          # matmul free-dim chunk
QC = QL // N     # 2 query chunks
LC = L // N      # 4 key free-dim chunks
LOG2E = 1.4426950408889634
SCALE = C ** -0.5
QSCALE = SCALE * 1024.0 * LOG2E      # folded into wq on the host
EXPSC = float(np.log(2.0) / 1024.0)  # ScalarE exp scale undoing QSCALE
BSHIFT = 15.0 * 1024.0 - 60.0        # Schraudolph fp16 exponent bias
EXP = mybir.ActivationFunctionType.Exp
IDENT = mybir.ActivationFunctionType.Identity
# track-B kt tiles whose exp runs (exactly) on ScalarE instead of the DVE;
# kt 15 is on ScalarE so the end-of-pair denominator tail starts fast.
SCALAR_B_KTS = (1, 5, 9, 15)
GP_B_KTS = (1, 5, 9)  # ScalarE-exp'd B tiles summed on GpSimd (minus kt 15)


def emit(nc, tc, x, wq, wk, wv, wo, bias, out):
    import contextlib
    ctx = contextlib.ExitStack()
    with ctx:
        # ---- pools -----------------------------------------------------
        consts = ctx.enter_context(tc.tile_pool(name="consts", bufs=1))
        qkv = ctx.enter_context(tc.tile_pool(name="qkv", bufs=1))
        ph1 = ctx.enter_context(tc.tile_pool(name="ph1", bufs=1))
        atAp = ctx.enter_context(tc.tile_pool(name="atAp", bufs=3))
        atBp = ctx.enter_context(tc.tile_pool(name="atBp", bufs=3))
        # ScalarE-exp'd B tiles live longer (read by the GpSimd accumulator
        # several kt later), so they get their own slots
        atBSp = ctx.enter_context(tc.tile_pool(name="atBSp", bufs=2))
        accVp = ctx.enter_context(tc.tile_pool(name="accVp", bufs=2))
        accGp = ctx.enter_context(tc.tile_pool(name="accGp", bufs=2))
        t1Bp = ctx.enter_context(tc.tile_pool(name="t1Bp", bufs=2))
        rcpp = ctx.enter_context(tc.tile_pool(name="rcpp", bufs=2))
        rcp16p = ctx.enter_context(tc.tile_pool(name="rcp16p", bufs=2))
        otup = ctx.enter_context(tc.tile_pool(name="otup", bufs=2))
        outp = ctx.enter_context(tc.tile_pool(name="outp", bufs=2))
        # PSUM: scores 2x[P,QL] (4 banks) + po [P,QL] (2) + den0/den1 (2)
        pps = ctx.enter_context(tc.tile_pool(name="pps", bufs=2, space="PSUM"))
        ppo = ctx.enter_context(tc.tile_pool(name="ppo", bufs=1, space="PSUM"))
        pd0 = ctx.enter_context(tc.tile_pool(name="pd0", bufs=1, space="PSUM"))
        pd1 = ctx.enter_context(tc.tile_pool(name="pd1", bufs=1, space="PSUM"))

        # ---- persistent SBUF ------------------------------------------
        wo_sb = consts.tile([P, DC, HID], F16)
        bias_sb = consts.tile([P, DC], F32)
        ones_sb = consts.tile([P, C], F16)
        nc.vector.memset(ones_sb[:], 1.0)

        q_sb = qkv.tile([P, DC, QL], F16)
        k_sb = qkv.tile([P, DC, L], F16)
        vt_sb = qkv.tile([P, KT, HID], F16)
        ot_sb = qkv.tile([P, DC, QL], F16)

        # ---- phase 1: load x + weights, QKV projections ----------------
        x_sb = ph1.tile([P, DC, L], F16)
        wq_sb = ph1.tile([P, DC, HID], F16)
        wk_sb = ph1.tile([P, DC, HID], F16)
        wv_sb = ph1.tile([P, DC, HID], F16)
        xr = x.rearrange("(a p) n -> p a n", p=P)
        # query-half of x + wq first so the Q projection starts ASAP
        nc.sync.dma_start(out=wq_sb[:], in_=wq.rearrange("(a p) n -> p a n", p=P))
        nc.sync.dma_start(out=x_sb[:, :, 0:N], in_=xr[:, :, 0:N])
        nc.sync.dma_start(out=x_sb[:, :, N:QL], in_=xr[:, :, N:QL])
        nc.sync.dma_start(out=wk_sb[:], in_=wk.rearrange("(a p) n -> p a n", p=P))
        nc.sync.dma_start(out=x_sb[:, :, QL:L], in_=xr[:, :, QL:L])
        nc.sync.dma_start(out=wv_sb[:], in_=wv.rearrange("(a p) n -> p a n", p=P))
        nc.sync.dma_start(out=wo_sb[:], in_=wo.rearrange("(a p) n -> p a n", p=P))
        nc.sync.dma_start(out=bias_sb[:], in_=bias)

        # phase-1 PSUM staging rotates through the den + po bank slots
        _prj = [0]

        def proj_psum():
            _prj[0] += 1
            i = _prj[0] % 3
            if i == 0:
                return pd0.tile([P, N], F32, tag="d0", name=f"prj{_prj[0]}")
            if i == 1:
                return pd1.tile([P, N], F32, tag="d1", name=f"prj{_prj[0]}")
            return ppo.tile([P, N], F32, tag="po", name=f"prj{_prj[0]}")

        cp = [0]

        def proj_copy(dst, src):
            # split PSUM->SBUF projection copies across ScalarE and DVE
            cp[0] += 1
            if cp[0] % 2 == 0:
                nc.scalar.copy(dst, src)
            else:
                nc.vector.tensor_copy(dst, src)

        # Q: [hc, q] = wq^T @ xq  (queries are x cols 0:QL)
        for m in range(DC):
            for qc in range(QC):
                ps = proj_psum()
                for dc in range(DC):
                    nc.tensor.matmul(
                        ps[:],
                        lhsT=wq_sb[:, dc, m * P:(m + 1) * P],
                        rhs=x_sb[:, dc, qc * N:(qc + 1) * N],
                        start=(dc == 0), stop=(dc == DC - 1),
                    )
                proj_copy(q_sb[:, m, qc * N:(qc + 1) * N], ps[:])

        def k_proj(m, lc):
            ps = proj_psum()
            for dc in range(DC):
                nc.tensor.matmul(
                    ps[:],
                    lhsT=wk_sb[:, dc, m * P:(m + 1) * P],
                    rhs=x_sb[:, dc, lc * N:(lc + 1) * N],
                    start=(dc == 0), stop=(dc == DC - 1),
                )
            proj_copy(k_sb[:, m, lc * N:(lc + 1) * N], ps[:])

        def vt_proj(kt):
            # V^T: [k, hc] (x stationary)
            ps = proj_psum()
            for dc in range(DC):
                nc.tensor.matmul(
                    ps[:],
                    lhsT=x_sb[:, dc, kt * P:(kt + 1) * P],
                    rhs=wv_sb[:, dc, :],
                    start=(dc == 0), stop=(dc == DC - 1),
                )
            proj_copy(vt_sb[:, kt, :], ps[:])

        # K and V interleaved by chunk so both become ready kt-progressively
        for lc in range(LC):
            for m in range(DC):
                k_proj(m, lc)
            for kt in range(4 * lc, 4 * lc + 4):
                vt_proj(kt)

        # ---- phase 2: attention, one head pair (2m, 2m+1) at a time ----
        for m in range(DC):
            po = ppo.tile([P, QL], F32, tag="po", name=f"po{m}")
            den0 = pd0.tile([P, N], F32, tag="d0", name=f"den0_{m}")
            den1 = pd1.tile([P, N], F32, tag="d1", name=f"den1_{m}")
            accV = accVp.tile([P, QL], F16, tag="accV", name=f"accV{m}")
            accG = accGp.tile([P, QL], F16, tag="accG", name=f"accG{m}")
            atB_prev = None   # first Schraudolph tile, consumed by 2nd
            atB_s1 = None     # ScalarE-exp'd B tile at kt=1
            atB_last = None   # kt=15 B tile (ScalarE), summed in the tail
            nV = 0
            for kt in range(KT):
                ps_A = pps.tile([P, QL], F32, tag="ps")
                ps_B = pps.tile([P, QL], F32, tag="ps")
                at_A = atAp.tile([P, QL], F16, tag="at")
                if kt in SCALAR_B_KTS:
                    atB = atBSp.tile([P, QL], I16, tag="atbs")
                else:
                    atB = atBp.tile([P, QL], I16, tag="atb")
                atB16 = atB[:].bitcast(F16)
                # scores: interleave row-packed pairs for PE concurrency
                for qc in range(QC):
                    nc.tensor.matmul(
                        ps_A[:, qc * N:(qc + 1) * N],
                        lhsT=k_sb[0:C, m, kt * P:(kt + 1) * P],
                        rhs=q_sb[0:C, m, qc * N:(qc + 1) * N],
                        start=True, stop=True, tile_position=(0, 0),
                    )
                    nc.tensor.matmul(
                        ps_B[:, qc * N:(qc + 1) * N],
                        lhsT=k_sb[C:P, m, kt * P:(kt + 1) * P],
                        rhs=q_sb[C:P, m, qc * N:(qc + 1) * N],
                        start=True, stop=True, tile_position=(C, 0),
                    )
                # exp: track A exact on ScalarE; track B mostly Schraudolph
                # on the DVE (int16 bit-trick), some tiles on ScalarE
                nc.scalar.activation(at_A[:], ps_A[:], EXP, scale=EXPSC)
                on_scalar = kt in SCALAR_B_KTS
                if on_scalar:
                    nc.scalar.activation(atB16, ps_B[:], EXP, scale=EXPSC)
                else:
                    nc.vector.tensor_scalar_add(atB[:], ps_B[:], BSHIFT)
                # attn @ V: col-packed pair, interleaved for concurrency;
                # accumulate over kt into disjoint partition halves.
                for qc in range(QC):
                    nc.tensor.matmul(
                        po[0:C, qc * N:(qc + 1) * N],
                        lhsT=vt_sb[:, kt, (2 * m) * C:(2 * m + 1) * C],
                        rhs=at_A[:, qc * N:(qc + 1) * N],
                        start=(kt == 0), stop=(kt == KT - 1),
                        tile_position=(0, 0), skip_group_check=True,
                    )
                    nc.tensor.matmul(
                        po[C:P, qc * N:(qc + 1) * N],
                        lhsT=vt_sb[:, kt, (2 * m + 1) * C:(2 * m + 2) * C],
                        rhs=atB16[:, qc * N:(qc + 1) * N],
                        start=(kt == 0), stop=(kt == KT - 1),
                        tile_position=(0, C), skip_group_check=True,
                    )
                # A denominator: ones-matmul accumulation (broadcast rows)
                for qc, den_t in ((0, den0), (1, den1)):
                    nc.tensor.matmul(
                        den_t[0:C, :],
                        lhsT=ones_sb[:],
                        rhs=at_A[:, qc * N:(qc + 1) * N],
                        start=(kt == 0), stop=(kt == KT - 1),
                        tile_position=(0, 0), skip_group_check=True,
                    )
                # B denominator: progressive accumulators.  DVE sums the
                # Schraudolph tiles; GpSimd sums the ScalarE tiles.
                if kt == KT - 1:
                    atB_last = atB16
                elif on_scalar:
                    if atB_s1 is None:
                        atB_s1 = atB16
                    elif kt == GP_B_KTS[1]:
                        nc.gpsimd.tensor_add(accG[:], atB_s1, atB16)
                    else:
                        nc.gpsimd.tensor_add(accG[:], accG[:], atB16)
                else:
                    if atB_prev is None:
                        atB_prev = atB16
                    elif nV == 0:
                        nc.vector.tensor_add(accV[:], atB_prev, atB16)
                        nV = 1
                    else:
                        nc.vector.tensor_add(accV[:], accV[:], atB16)

            # ---- end-of-pair tail ------------------------------------
            # B denominator: t1 = accV + accG + at(15), then ones-matmul
            # partition-sum into den banks at partitions 64:128.
            t1a = t1Bp.tile([P, QL], F16, tag="t1", name=f"t1a_{m}")
            t1b = t1Bp.tile([P, QL], F16, tag="t1", name=f"t1b_{m}")
            nc.vector.tensor_add(t1a[:], accV[:], accG[:])
            nc.vector.tensor_add(t1b[:], t1a[:], atB_last)
            for qc, den_t in ((0, den0), (1, den1)):
                nc.tensor.matmul(
                    den_t[C:P, :],
                    lhsT=ones_sb[:],
                    rhs=t1b[:, qc * N:(qc + 1) * N],
                    start=True, stop=True,
                    tile_position=(0, C), skip_group_check=True,
                )
            # free the po banks quickly, then normalize
            otu = otup.tile([P, QL], F16, tag="otu", name=f"otu{m}")
            nc.scalar.copy(otu[:, 0:N], po[:, 0:N])
            nc.vector.tensor_copy(otu[:, N:QL], po[:, N:QL])
            for qc, den_t in ((0, den0), (1, den1)):
                rc32 = rcpp.tile([P, N], F32, tag="rc32", name=f"rc32_{m}{qc}")
                nc.vector.reciprocal_approx_fast(out=rc32[:], in_=den_t[:])
                rc16 = rcp16p.tile([P, N], F16, tag="rc16", name=f"rc16_{m}{qc}")
                nc.vector.tensor_copy(rc16[:], rc32[:])
                for half in range(2):
                    nc.vector.tensor_mul(
                        ot_sb[half * C:(half + 1) * C, m, qc * N:(qc + 1) * N],
                        otu[half * C:(half + 1) * C, qc * N:(qc + 1) * N],
                        rc16[half * C:(half + 1) * C, :],
                    )

        # ---- phase 3: output projection + bias -------------------------
        for mo in range(DC):
            for qc in range(QC):
                ps = pps.tile([P, N], F32, tag="ps", name=f"po3_{mo}_{qc}")
                for mh in range(DC):
                    nc.tensor.matmul(
                        ps[:],
                        lhsT=wo_sb[:, mh, mo * P:(mo + 1) * P],
                        rhs=ot_sb[:, mh, qc * N:(qc + 1) * N],
                        start=(mh == 0), stop=(mh == DC - 1),
                    )
                ob = outp.tile([P, N], F32, tag="ob")
                if (mo + qc) % 2 == 0:
                    nc.vector.tensor_scalar_add(ob[:], ps[:], bias_sb[:, mo:mo + 1])
                else:
                    nc.scalar.activation(ob[:], ps[:], IDENT,
                                         bias=bias_sb[:, mo:mo + 1])
                nc.sync.dma_start(
                    out=out[mo * P:(mo + 1) * P, qc * N:(qc + 1) * N], in_=ob[:]
                )


def build():
    nc = bacc.Bacc("TRN2", target_bir_lowering=False, debug=False)
    x = nc.dram_tensor("x", [D, L], F16, kind="ExternalInput").ap()
    wq = nc.dram_tensor("wq", [D, HID], F16, kind="ExternalInput").ap()
    wk = nc.dram_tensor("wk", [D, HID], F16, kind="ExternalInput").ap()
    wv = nc.dram_tensor("wv", [D, HID], F16, kind="ExternalInput").ap()
    wo = nc.dram_tensor("wo", [HID, D], F16, kind="ExternalInput").ap()
    bias = nc.dram_tensor("bias", [P, DC], F32, kind="ExternalInput").ap()
    out = nc.dram_tensor("out", [D, QL], F32, kind="ExternalOutput").ap()
    with tile.TileContext(nc) as tc:
        emit(nc, tc, x, wq, wk, wv, wo, bias, out)
    nc.compile()
    return nc


_NC_CACHE = None


def _get_nc():
    global _NC_CACHE
    if _NC_CACHE is None:
        _NC_CACHE = build()
    return _NC_CACHE


def make_in_maps(x, w_qkv, w_out, b_out):
    """Host-side sharding: returns the 8 per-core input dicts."""
    f16 = np.float16
    wq_t = np.ascontiguousarray((w_qkv[0:HID] * QSCALE).T).astype(f16)
    wk_t = np.ascontiguousarray(w_qkv[HID:2 * HID].T).astype(f16)
    wv_t = np.ascontiguousarray(w_qkv[2 * HID:3 * HID].T).astype(f16)
    wo_t = np.ascontiguousarray(w_out.T).astype(f16)
    bias = np.ascontiguousarray(b_out.reshape(DC, P).T).astype(np.float32)
    in_maps = []
    for core in range(8):
        b, halfq = core // 2, core % 2
        # rotate so this core's query half sits at columns 0:QL; key order
        # is irrelevant (softmax sums over all keys).
        x_rot = np.roll(x[b], -halfq * QL, axis=1).astype(f16)
        in_maps.append({
            "x": np.ascontiguousarray(x_rot),
            "wq": wq_t, "wk": wk_t, "wv": wv_t, "wo": wo_t,
            "bias": bias,
        })
    return in_maps


def assemble(results):
    out = np.zeros((4, D, L), np.float32)
    for core in range(8):
        b, halfq = core // 2, core % 2
        out[b][:, halfq * QL:(halfq + 1) * QL] = results[core]["out"]
    return out


def kernel(x, w_qkv, w_out, b_out):
    x = np.asarray(x, np.float32)
    w_qkv = np.asarray(w_qkv, np.float32)
    w_out = np.asarray(w_out, np.float32)
    b_out = np.asarray(b_out, np.float32)
    nc = _get_nc()
    in_maps = make_in_maps(x, w_qkv, w_out, b_out)
    res = run_bass_kernel_spmd(nc, in_maps, core_ids=list(range(8)))
    return assemble(res.results)
